# revision 35
# baseline (speedup 1.0000x reference)
"""Trainium2 Bass kernel for the BaseLTI Kalman filter problem.

Math: with mask == 1 everywhere and a batch-shared initial covariance, the
covariance recursion (innovation cholesky LS_t, Kalman gain K_t, filtered
cholesky L_t) is identical for every batch element, so it is computed once on
the host in float64.  The per-batch mean recursion is linear in y:

    mu_f(t)     = mu_pred(t) + K_t (y_t - H mu_pred(t))
    sol(t)      = LS_t^{-1} (y_t - H mu_pred(t))          (whitened innovation)
    mu_pred(t+1)= E mu_f(t),   E = (I + STEP*F)^2

so K=4 consecutive time steps collapse into one matmul over the augmented
input  X_c = [mu_pred ; y_t ; y_t+1 ; y_t+2 ; y_t+3]  (96 x batch).  The
device runs the 32-chunk sequential scan (data-parallel over batch: 64 batch
elements per core x 8 cores) and broadcasts the batch-independent L_t into
the (T, B, 32, 32) output, which dominates the kernel at ~33.5 MB of DMA
writes per core.

log_prob = C - 0.5 * sum_{t,y} sol^2 ; the device returns the per-batch sum
of squares, the constant C is applied on the host.
"""

import numpy as np

B, T, YD, ZD = 512, 128, 16, 32
STEP = 0.05
N_EULER = 2
NCORES = 8
BC = B // NCORES          # 64 batch per core
K = 4                     # time steps per chunk
NCHUNK = T // K           # 32
D = ZD + K * YD           # 96 contraction dim
AUXC = K * ZD + K * YD    # 192 aux matmul cols (mus | sol)

_compiled = {}


# ---------------------------------------------------------------- host math
def _host_precompute(F, H, qdiag, rdiag, sig0diag):
    """Batch-independent covariance recursion in float64."""
    F = F.astype(np.float64)
    H = H.astype(np.float64)
    Q = np.diag(qdiag.astype(np.float64))
    R = np.diag(rdiag.astype(np.float64))
    Iz = np.eye(ZD)
    E1 = Iz + STEP * F
    E = E1 @ E1
    Sig_pred = np.diag(sig0diag.astype(np.float64))
    Ls = np.zeros((T, ZD, ZD))
    Ks = np.zeros((T, ZD, YD))
    LSinvs = np.zeros((T, YD, YD))
    cts = np.zeros(T)
    LOG2PI = np.log(2.0 * np.pi)
    for t in range(T):
        S = H @ Sig_pred @ H.T + R
        LS = np.linalg.cholesky(S)
        PHt = Sig_pred @ H.T
        Kt = np.linalg.solve(S, PHt.T).T
        A = Iz - Kt @ H
        Sig_u = A @ Sig_pred @ A.T + Kt @ R @ Kt.T
        Ls[t] = np.linalg.cholesky(0.5 * (Sig_u + Sig_u.T))
        Ks[t] = Kt
        LSinvs[t] = np.linalg.inv(LS)
        cts[t] = -0.5 * YD * LOG2PI - np.sum(np.log(np.diag(LS)))
        Sig = Sig_u
        for _ in range(N_EULER):
            Sig = Sig + STEP * (F @ Sig + Sig @ F.T + Q)
        Sig_pred = 0.5 * (Sig + Sig.T)

    Wcrit = np.zeros((NCHUNK, D, ZD), np.float32)
    Waux = np.zeros((NCHUNK, D, AUXC), np.float32)
    for c in range(NCHUNK):
        state_map = np.zeros((ZD, D))
        state_map[:, :ZD] = Iz
        mus_maps, sol_maps = [], []
        for j in range(K):
            t = c * K + j
            ymap = np.zeros((YD, D))
            ymap[:, ZD + j * YD: ZD + (j + 1) * YD] = np.eye(YD)
            innov_map = ymap - H @ state_map
            mus_map = state_map + Ks[t] @ innov_map
            sol_maps.append(LSinvs[t] @ innov_map)
            mus_maps.append(mus_map)
            state_map = E @ mus_map
        Wcrit[c] = state_map.T.astype(np.float32)
        Waux[c] = np.concatenate(mus_maps + sol_maps, axis=0).T.astype(np.float32)
    return Ls.astype(np.float32), Wcrit, Waux, float(np.sum(cts))


def _reference_fallback(y, mask, F, H, qdiag, rdiag, mu0, sig0diag):
    """Exact per-batch recursion (float64) for the general-mask case."""
    Bsz = y.shape[0]
    F = F.astype(np.float64); H = H.astype(np.float64)
    Q = np.diag(qdiag.astype(np.float64)); R = np.diag(rdiag.astype(np.float64))
    Iz = np.eye(ZD)
    LOG2PI = np.log(2.0 * np.pi)
    mu_pred = np.broadcast_to(mu0.astype(np.float64), (Bsz, ZD)).copy()
    L_pred = np.broadcast_to(np.diag(np.sqrt(sig0diag.astype(np.float64))),
                             (Bsz, ZD, ZD)).copy()
    mus = np.zeros((T, Bsz, ZD)); Ls = np.zeros((T, Bsz, ZD, ZD))
    logp = np.zeros(Bsz)
    for t in range(T):
        y_i = y[:, t].astype(np.float64); m = mask[:, t].astype(np.float64)
        Sig_pred = L_pred @ np.swapaxes(L_pred, -1, -2)
        S = H @ Sig_pred @ H.T + R
        LS = np.linalg.cholesky(S)
        y_hat = mu_pred @ H.T
        PHt = Sig_pred @ H.T
        Kt = np.swapaxes(np.linalg.solve(S, np.swapaxes(PHt, -1, -2)), -1, -2)
        innov = y_i - y_hat
        mu_u = mu_pred + np.einsum('bzy,by->bz', Kt, innov)
        A = Iz - Kt @ H
        Sig_u = A @ Sig_pred @ np.swapaxes(A, -1, -2) + Kt @ R @ np.swapaxes(Kt, -1, -2)
        L_u = np.linalg.cholesky(0.5 * (Sig_u + np.swapaxes(Sig_u, -1, -2)))
        mu = m[:, None] * mu_u + (1 - m[:, None]) * mu_pred
        L = m[:, None, None] * L_u + (1 - m[:, None, None]) * L_pred
        sol = np.linalg.solve(LS, innov[..., None])[..., 0]
        logp += (-0.5 * YD * LOG2PI
                 - np.sum(np.log(np.diagonal(LS, axis1=-2, axis2=-1)), -1)
                 - 0.5 * np.sum(sol * sol, -1)) * m
        mus[t] = mu; Ls[t] = L
        Sig = L @ np.swapaxes(L, -1, -2)
        mu_n = mu
        for _ in range(N_EULER):
            mu_n = mu_n + STEP * (mu_n @ F.T)
            Sig = Sig + STEP * (F @ Sig + Sig @ F.T + Q)
        mu_pred = mu_n
        L_pred = np.linalg.cholesky(0.5 * (Sig + np.swapaxes(Sig, -1, -2)))
    return (mus.astype(np.float32), Ls.astype(np.float32), logp.astype(np.float32))


# ---------------------------------------------------------------- device
def _build_nc():
    import concourse.bacc as bacc
    import concourse.mybir as mybir
    import concourse.tile as tile

    f32 = mybir.dt.float32
    nc = bacc.Bacc("TRN2", target_bir_lowering=False, debug=False,
                   num_devices=NCORES)

    ybuf = nc.dram_tensor("ybuf", [K * YD, NCHUNK * BC], f32, kind="ExternalInput").ap()
    mu0b = nc.dram_tensor("mu0b", [ZD, BC], f32, kind="ExternalInput").ap()
    wcrit = nc.dram_tensor("wcrit", [D, NCHUNK * ZD], f32, kind="ExternalInput").ap()
    waux = nc.dram_tensor("waux", [D, NCHUNK * AUXC], f32, kind="ExternalInput").ap()
    lsmall = nc.dram_tensor("lsmall", [T, ZD * ZD], f32, kind="ExternalInput").ap()

    ls_out = nc.dram_tensor("ls_out", [T, BC, ZD * ZD], f32, kind="ExternalOutput").ap()
    mus_out = nc.dram_tensor("mus_out", [BC, T * ZD], f32, kind="ExternalOutput").ap()
    ssq_out = nc.dram_tensor("ssq_out", [BC, 1], f32, kind="ExternalOutput").ap()

    with tile.TileContext(nc) as tc:
        with (
            tc.tile_pool(name="const", bufs=1) as constp,
            tc.tile_pool(name="work", bufs=3) as workp,
            tc.tile_pool(name="psumS", bufs=3, space="PSUM") as psumSp,
            tc.tile_pool(name="psumA", bufs=3, space="PSUM") as psumAp,
        ):
            X = constp.tile([D, NCHUNK * BC], f32)
            wc = constp.tile([D, NCHUNK * ZD], f32)
            wa = constp.tile([D, NCHUNK * AUXC], f32)
            ls = constp.tile([T, ZD * ZD], f32)
            musAll = constp.tile([BC, T * ZD], f32)
            acc = constp.tile([BC, K * YD], f32)

            # Broadcast-source loads on the Sync queue right before the
            # broadcasts; scan-critical loads on the Scalar engine's HWDGE
            # queue so they are not FIFO-ordered behind the 33.5 MB
            # broadcast stream.  Inputs are split into smaller DMAs: a
            # single large DMA drains slowly (~100 GB/s) and its completion
            # semaphore stalls later DMAs in the shared rotation.
            # HWDGE carries ONLY the ls loads + broadcasts: any other DMA in
            # the shared 9-sem rotation delays the broadcast stream start
            # (observed: ls loads rotation-stuck behind input loads until
            # ~21us).  All scan inputs go via SWDGE (own sem pool); the scan
            # has ~40us of slack under the broadcast stream.
            NG = 4
            TG = T // NG
            for g in range(NG):
                nc.sync.dma_start(ls[g * TG:(g + 1) * TG, :],
                                  lsmall[g * TG:(g + 1) * TG, :])
            half = NCHUNK * BC // 2
            nc.gpsimd.dma_start(X[ZD:D, 0:half], ybuf[:, 0:half])
            nc.gpsimd.dma_start(X[0:ZD, 0:BC], mu0b)
            nc.gpsimd.dma_start(X[ZD:D, half:2 * half], ybuf[:, half:2 * half])
            whalf = NCHUNK * ZD // 2
            nc.gpsimd.dma_start(wc[:, 0:whalf], wcrit[:, 0:whalf])
            nc.gpsimd.dma_start(wc[:, whalf:2 * whalf], wcrit[:, whalf:2 * whalf])
            # waux on SWDGE: keeps the HWDGE sem rotation clear of slow input
            # completions so the broadcast stream starts without stalls; wa
            # blocks are consumed progressively by the scan, so SWDGE's
            # lower bandwidth is hidden.
            NWA = 12
            wag = NCHUNK * AUXC // NWA
            for g in range(NWA):
                nc.gpsimd.dma_start(wa[:, g * wag:(g + 1) * wag],
                                    waux[:, g * wag:(g + 1) * wag])
            nc.vector.memset(acc[:], 0.0)

            # dominant work: broadcast L_t over the batch dim of ls_out.
            # One dma_start per batch element: separate ring entries fan out
            # across the DMA engines (a single big step-0 DMA runs ~3x
            # slower; per-engine streams cap at ~23 GB/s).
            for b in range(BC):
                nc.sync.dma_start(ls_out[:, b, :], ls[:])

            MCG = 4  # chunks per incremental mus_out DMA
            for c in range(NCHUNK):
                xblk = X[:, c * BC:(c + 1) * BC]
                if c < NCHUNK - 1:
                    ps = psumSp.tile([ZD, BC], f32)
                    nc.tensor.matmul(ps[:], wc[:, c * ZD:(c + 1) * ZD], xblk,
                                     start=True, stop=True)
                    nc.vector.tensor_copy(X[0:ZD, (c + 1) * BC:(c + 2) * BC], ps[:])
                pa = psumAp.tile([BC, AUXC], f32)
                nc.tensor.matmul(pa[:], xblk, wa[:, c * AUXC:(c + 1) * AUXC],
                                 start=True, stop=True)
                nc.scalar.copy(musAll[:, c * K * ZD:(c + 1) * K * ZD], pa[:, 0:K * ZD])
                sq = workp.tile([BC, K * YD], f32)
                nc.scalar.square(sq[:], pa[:, K * ZD:AUXC])
                nc.vector.tensor_add(acc[:], acc[:], sq[:])
                if c % MCG == MCG - 1:
                    # SWDGE: separate semaphore pool, so these mid-stream
                    # writes never stall the HWDGE broadcast rotation.
                    cols = slice((c - MCG + 1) * K * ZD, (c + 1) * K * ZD)
                    nc.gpsimd.dma_start(mus_out[:, cols], musAll[:, cols])
            red = workp.tile([BC, 1], f32)
            nc.vector.reduce_sum(red[:], acc[:], axis=mybir.AxisListType.X)
            nc.gpsimd.dma_start(ssq_out, red[:])

    nc.compile()
    return nc


def _get_nc():
    if "nc" not in _compiled:
        _compiled["nc"] = _build_nc()
    return _compiled["nc"]


LAST_RESULTS = None  # BassKernelResults of the most recent device run


def kernel(y, mask, times, F, H, qdiag, rdiag, mu0, sig0diag):
    global LAST_RESULTS
    y = np.ascontiguousarray(np.asarray(y, np.float32))
    mask = np.asarray(mask, np.float32)
    F = np.asarray(F); H = np.asarray(H)
    qdiag = np.asarray(qdiag); rdiag = np.asarray(rdiag)
    mu0 = np.asarray(mu0); sig0diag = np.asarray(sig0diag)

    if not np.all(mask == 1.0):
        return _reference_fallback(y, mask, F, H, qdiag, rdiag, mu0, sig0diag)

    from concourse.bass_utils import run_bass_kernel_spmd

    Ls, Wcrit, Waux, Ctot = _host_precompute(F, H, qdiag, rdiag, sig0diag)
    wcritA = np.ascontiguousarray(Wcrit.transpose(1, 0, 2).reshape(D, NCHUNK * ZD))
    wauxA = np.ascontiguousarray(Waux.transpose(1, 0, 2).reshape(D, NCHUNK * AUXC))
    lsA = np.ascontiguousarray(Ls.reshape(T, ZD * ZD))
    mu0bA = np.ascontiguousarray(
        np.broadcast_to(mu0.astype(np.float32)[:, None], (ZD, BC)))

    in_maps = []
    for ci in range(NCORES):
        yc = y[ci * BC:(ci + 1) * BC]                       # (BC, T, YD)
        ybuf = np.ascontiguousarray(
            yc.reshape(BC, NCHUNK, K * YD).transpose(2, 1, 0)
            .reshape(K * YD, NCHUNK * BC))                  # X y-rows layout
        in_maps.append(dict(ybuf=ybuf, mu0b=mu0bA, wcrit=wcritA,
                            waux=wauxA, lsmall=lsA))

    nc = _get_nc()
    res = run_bass_kernel_spmd(nc, in_maps, core_ids=list(range(NCORES)))
    LAST_RESULTS = res

    ls_full = np.concatenate(
        [r["ls_out"].reshape(T, BC, ZD, ZD) for r in res.results], axis=1)
    mus_full = np.concatenate(
        [r["mus_out"].reshape(BC, T, ZD) for r in res.results], axis=0
    ).transpose(1, 0, 2)
    ssq = np.concatenate([r["ssq_out"][:, 0] for r in res.results], axis=0)
    logp = (Ctot - 0.5 * ssq.astype(np.float64)).astype(np.float32)
    return mus_full, ls_full, logp


# revision 36
# speedup vs baseline: 1.0696x; 1.0696x over previous
"""Trainium2 Bass kernel for the BaseLTI Kalman filter problem.

Math: with mask == 1 everywhere and a batch-shared initial covariance, the
covariance recursion (innovation cholesky LS_t, Kalman gain K_t, filtered
cholesky L_t) is identical for every batch element, so it is computed once on
the host in float64.  The per-batch mean recursion is linear in y:

    mu_f(t)     = mu_pred(t) + K_t (y_t - H mu_pred(t))
    sol(t)      = LS_t^{-1} (y_t - H mu_pred(t))          (whitened innovation)
    mu_pred(t+1)= E mu_f(t),   E = (I + STEP*F)^2

so K=4 consecutive time steps collapse into one matmul over the augmented
input  X_c = [mu_pred ; y_t ; y_t+1 ; y_t+2 ; y_t+3]  (96 x batch).  The
device runs the 32-chunk sequential scan (data-parallel over batch: 64 batch
elements per core x 8 cores) and broadcasts the batch-independent L_t into
the (T, B, 32, 32) output, which dominates the kernel at ~33.5 MB of DMA
writes per core.

log_prob = C - 0.5 * sum_{t,y} sol^2 ; the device returns the per-batch sum
of squares, the constant C is applied on the host.
"""

import numpy as np

B, T, YD, ZD = 512, 128, 16, 32
STEP = 0.05
N_EULER = 2
NCORES = 8
BC = B // NCORES          # 64 batch per core
K = 4                     # time steps per chunk
NCHUNK = T // K           # 32
D = ZD + K * YD           # 96 contraction dim
AUXC = K * ZD + K * YD    # 192 aux matmul cols (mus | sol)

_compiled = {}


# ---------------------------------------------------------------- host math
def _host_precompute(F, H, qdiag, rdiag, sig0diag):
    """Batch-independent covariance recursion in float64."""
    F = F.astype(np.float64)
    H = H.astype(np.float64)
    Q = np.diag(qdiag.astype(np.float64))
    R = np.diag(rdiag.astype(np.float64))
    Iz = np.eye(ZD)
    E1 = Iz + STEP * F
    E = E1 @ E1
    Sig_pred = np.diag(sig0diag.astype(np.float64))
    Ls = np.zeros((T, ZD, ZD))
    Ks = np.zeros((T, ZD, YD))
    LSinvs = np.zeros((T, YD, YD))
    cts = np.zeros(T)
    LOG2PI = np.log(2.0 * np.pi)
    for t in range(T):
        S = H @ Sig_pred @ H.T + R
        LS = np.linalg.cholesky(S)
        PHt = Sig_pred @ H.T
        Kt = np.linalg.solve(S, PHt.T).T
        A = Iz - Kt @ H
        Sig_u = A @ Sig_pred @ A.T + Kt @ R @ Kt.T
        Ls[t] = np.linalg.cholesky(0.5 * (Sig_u + Sig_u.T))
        Ks[t] = Kt
        LSinvs[t] = np.linalg.inv(LS)
        cts[t] = -0.5 * YD * LOG2PI - np.sum(np.log(np.diag(LS)))
        Sig = Sig_u
        for _ in range(N_EULER):
            Sig = Sig + STEP * (F @ Sig + Sig @ F.T + Q)
        Sig_pred = 0.5 * (Sig + Sig.T)

    Wcrit = np.zeros((NCHUNK, D, ZD), np.float32)
    Waux = np.zeros((NCHUNK, D, AUXC), np.float32)
    for c in range(NCHUNK):
        state_map = np.zeros((ZD, D))
        state_map[:, :ZD] = Iz
        mus_maps, sol_maps = [], []
        for j in range(K):
            t = c * K + j
            ymap = np.zeros((YD, D))
            ymap[:, ZD + j * YD: ZD + (j + 1) * YD] = np.eye(YD)
            innov_map = ymap - H @ state_map
            mus_map = state_map + Ks[t] @ innov_map
            sol_maps.append(LSinvs[t] @ innov_map)
            mus_maps.append(mus_map)
            state_map = E @ mus_map
        Wcrit[c] = state_map.T.astype(np.float32)
        Waux[c] = np.concatenate(mus_maps + sol_maps, axis=0).T.astype(np.float32)
    return Ls.astype(np.float32), Wcrit, Waux, float(np.sum(cts))


def _reference_fallback(y, mask, F, H, qdiag, rdiag, mu0, sig0diag):
    """Exact per-batch recursion (float64) for the general-mask case."""
    Bsz = y.shape[0]
    F = F.astype(np.float64); H = H.astype(np.float64)
    Q = np.diag(qdiag.astype(np.float64)); R = np.diag(rdiag.astype(np.float64))
    Iz = np.eye(ZD)
    LOG2PI = np.log(2.0 * np.pi)
    mu_pred = np.broadcast_to(mu0.astype(np.float64), (Bsz, ZD)).copy()
    L_pred = np.broadcast_to(np.diag(np.sqrt(sig0diag.astype(np.float64))),
                             (Bsz, ZD, ZD)).copy()
    mus = np.zeros((T, Bsz, ZD)); Ls = np.zeros((T, Bsz, ZD, ZD))
    logp = np.zeros(Bsz)
    for t in range(T):
        y_i = y[:, t].astype(np.float64); m = mask[:, t].astype(np.float64)
        Sig_pred = L_pred @ np.swapaxes(L_pred, -1, -2)
        S = H @ Sig_pred @ H.T + R
        LS = np.linalg.cholesky(S)
        y_hat = mu_pred @ H.T
        PHt = Sig_pred @ H.T
        Kt = np.swapaxes(np.linalg.solve(S, np.swapaxes(PHt, -1, -2)), -1, -2)
        innov = y_i - y_hat
        mu_u = mu_pred + np.einsum('bzy,by->bz', Kt, innov)
        A = Iz - Kt @ H
        Sig_u = A @ Sig_pred @ np.swapaxes(A, -1, -2) + Kt @ R @ np.swapaxes(Kt, -1, -2)
        L_u = np.linalg.cholesky(0.5 * (Sig_u + np.swapaxes(Sig_u, -1, -2)))
        mu = m[:, None] * mu_u + (1 - m[:, None]) * mu_pred
        L = m[:, None, None] * L_u + (1 - m[:, None, None]) * L_pred
        sol = np.linalg.solve(LS, innov[..., None])[..., 0]
        logp += (-0.5 * YD * LOG2PI
                 - np.sum(np.log(np.diagonal(LS, axis1=-2, axis2=-1)), -1)
                 - 0.5 * np.sum(sol * sol, -1)) * m
        mus[t] = mu; Ls[t] = L
        Sig = L @ np.swapaxes(L, -1, -2)
        mu_n = mu
        for _ in range(N_EULER):
            mu_n = mu_n + STEP * (mu_n @ F.T)
            Sig = Sig + STEP * (F @ Sig + Sig @ F.T + Q)
        mu_pred = mu_n
        L_pred = np.linalg.cholesky(0.5 * (Sig + np.swapaxes(Sig, -1, -2)))
    return (mus.astype(np.float32), Ls.astype(np.float32), logp.astype(np.float32))


# ---------------------------------------------------------------- device
def _build_nc():
    import concourse.bacc as bacc
    import concourse.mybir as mybir
    import concourse.tile as tile

    f32 = mybir.dt.float32
    nc = bacc.Bacc("TRN2", target_bir_lowering=False, debug=False,
                   num_devices=NCORES)

    ybuf = nc.dram_tensor("ybuf", [K * YD, NCHUNK * BC], f32, kind="ExternalInput").ap()
    mu0b = nc.dram_tensor("mu0b", [ZD, BC], f32, kind="ExternalInput").ap()
    wcrit = nc.dram_tensor("wcrit", [D, NCHUNK * ZD], f32, kind="ExternalInput").ap()
    waux = nc.dram_tensor("waux", [D, NCHUNK * AUXC], f32, kind="ExternalInput").ap()
    lsmall = nc.dram_tensor("lsmall", [T, ZD * ZD], f32, kind="ExternalInput").ap()

    ls_out = nc.dram_tensor("ls_out", [T, BC, ZD * ZD], f32, kind="ExternalOutput").ap()
    mus_out = nc.dram_tensor("mus_out", [BC, T * ZD], f32, kind="ExternalOutput").ap()
    ssq_out = nc.dram_tensor("ssq_out", [BC, 1], f32, kind="ExternalOutput").ap()

    with tile.TileContext(nc) as tc:
        with (
            tc.tile_pool(name="const", bufs=1) as constp,
            tc.tile_pool(name="work", bufs=3) as workp,
            tc.tile_pool(name="psumS", bufs=3, space="PSUM") as psumSp,
            tc.tile_pool(name="psumA", bufs=3, space="PSUM") as psumAp,
        ):
            X = constp.tile([D, NCHUNK * BC], f32)
            wc = constp.tile([D, NCHUNK * ZD], f32)
            wa = constp.tile([D, NCHUNK * AUXC], f32)
            ls = constp.tile([T, ZD * ZD], f32)
            musAll = constp.tile([BC, T * ZD], f32)
            acc = constp.tile([BC, K * YD], f32)

            # Broadcast-source loads on the Sync queue right before the
            # broadcasts; scan-critical loads on the Scalar engine's HWDGE
            # queue so they are not FIFO-ordered behind the 33.5 MB
            # broadcast stream.  Inputs are split into smaller DMAs: a
            # single large DMA drains slowly (~100 GB/s) and its completion
            # semaphore stalls later DMAs in the shared rotation.
            NG = 8
            TG = T // NG
            for g in range(NG):
                nc.sync.dma_start(ls[g * TG:(g + 1) * TG, :],
                                  lsmall[g * TG:(g + 1) * TG, :])
            half = NCHUNK * BC // 2
            nc.scalar.dma_start(X[ZD:D, 0:half], ybuf[:, 0:half])
            nc.scalar.dma_start(X[0:ZD, 0:BC], mu0b)
            nc.scalar.dma_start(X[ZD:D, half:2 * half], ybuf[:, half:2 * half])
            whalf = NCHUNK * ZD // 2
            nc.scalar.dma_start(wc[:, 0:whalf], wcrit[:, 0:whalf])
            nc.scalar.dma_start(wc[:, whalf:2 * whalf], wcrit[:, whalf:2 * whalf])
            # waux on SWDGE: keeps the HWDGE sem rotation clear of slow input
            # completions so the broadcast stream starts without stalls; wa
            # blocks are consumed progressively by the scan, so SWDGE's
            # lower bandwidth is hidden.
            NWA = 12
            wag = NCHUNK * AUXC // NWA
            for g in range(NWA):
                nc.gpsimd.dma_start(wa[:, g * wag:(g + 1) * wag],
                                    waux[:, g * wag:(g + 1) * wag])
            nc.vector.memset(acc[:], 0.0)

            # dominant work: broadcast L_t over the batch dim of ls_out.
            # One dma_start per batch element: separate ring entries fan out
            # across the DMA engines (a single big step-0 DMA runs ~3x
            # slower; per-engine streams cap at ~23 GB/s).
            for b in range(BC):
                nc.sync.dma_start(ls_out[:, b, :], ls[:])

            MCG = 4  # chunks per incremental mus_out DMA
            for c in range(NCHUNK):
                xblk = X[:, c * BC:(c + 1) * BC]
                if c < NCHUNK - 1:
                    ps = psumSp.tile([ZD, BC], f32)
                    nc.tensor.matmul(ps[:], wc[:, c * ZD:(c + 1) * ZD], xblk,
                                     start=True, stop=True)
                    nc.vector.tensor_copy(X[0:ZD, (c + 1) * BC:(c + 2) * BC], ps[:])
                pa = psumAp.tile([BC, AUXC], f32)
                nc.tensor.matmul(pa[:], xblk, wa[:, c * AUXC:(c + 1) * AUXC],
                                 start=True, stop=True)
                nc.scalar.copy(musAll[:, c * K * ZD:(c + 1) * K * ZD], pa[:, 0:K * ZD])
                sq = workp.tile([BC, K * YD], f32)
                nc.scalar.square(sq[:], pa[:, K * ZD:AUXC])
                nc.vector.tensor_add(acc[:], acc[:], sq[:])
                if c % MCG == MCG - 1:
                    # SWDGE: separate semaphore pool, so these mid-stream
                    # writes never stall the HWDGE broadcast rotation.
                    cols = slice((c - MCG + 1) * K * ZD, (c + 1) * K * ZD)
                    nc.gpsimd.dma_start(mus_out[:, cols], musAll[:, cols])
            red = workp.tile([BC, 1], f32)
            nc.vector.reduce_sum(red[:], acc[:], axis=mybir.AxisListType.X)
            nc.gpsimd.dma_start(ssq_out, red[:])

    nc.compile()
    return nc


def _get_nc():
    if "nc" not in _compiled:
        _compiled["nc"] = _build_nc()
    return _compiled["nc"]


LAST_RESULTS = None  # BassKernelResults of the most recent device run


def kernel(y, mask, times, F, H, qdiag, rdiag, mu0, sig0diag):
    global LAST_RESULTS
    y = np.ascontiguousarray(np.asarray(y, np.float32))
    mask = np.asarray(mask, np.float32)
    F = np.asarray(F); H = np.asarray(H)
    qdiag = np.asarray(qdiag); rdiag = np.asarray(rdiag)
    mu0 = np.asarray(mu0); sig0diag = np.asarray(sig0diag)

    if not np.all(mask == 1.0):
        return _reference_fallback(y, mask, F, H, qdiag, rdiag, mu0, sig0diag)

    from concourse.bass_utils import run_bass_kernel_spmd

    Ls, Wcrit, Waux, Ctot = _host_precompute(F, H, qdiag, rdiag, sig0diag)
    wcritA = np.ascontiguousarray(Wcrit.transpose(1, 0, 2).reshape(D, NCHUNK * ZD))
    wauxA = np.ascontiguousarray(Waux.transpose(1, 0, 2).reshape(D, NCHUNK * AUXC))
    lsA = np.ascontiguousarray(Ls.reshape(T, ZD * ZD))
    mu0bA = np.ascontiguousarray(
        np.broadcast_to(mu0.astype(np.float32)[:, None], (ZD, BC)))

    in_maps = []
    for ci in range(NCORES):
        yc = y[ci * BC:(ci + 1) * BC]                       # (BC, T, YD)
        ybuf = np.ascontiguousarray(
            yc.reshape(BC, NCHUNK, K * YD).transpose(2, 1, 0)
            .reshape(K * YD, NCHUNK * BC))                  # X y-rows layout
        in_maps.append(dict(ybuf=ybuf, mu0b=mu0bA, wcrit=wcritA,
                            waux=wauxA, lsmall=lsA))

    nc = _get_nc()
    res = run_bass_kernel_spmd(nc, in_maps, core_ids=list(range(NCORES)))
    LAST_RESULTS = res

    ls_full = np.concatenate(
        [r["ls_out"].reshape(T, BC, ZD, ZD) for r in res.results], axis=1)
    mus_full = np.concatenate(
        [r["mus_out"].reshape(BC, T, ZD) for r in res.results], axis=0
    ).transpose(1, 0, 2)
    ssq = np.concatenate([r["ssq_out"][:, 0] for r in res.results], axis=0)
    logp = (Ctot - 0.5 * ssq.astype(np.float64)).astype(np.float32)
    return mus_full, ls_full, logp


# revision 37
# speedup vs baseline: 1.0826x; 1.0122x over previous
"""Trainium2 Bass kernel for the BaseLTI Kalman filter problem.

Math: with mask == 1 everywhere and a batch-shared initial covariance, the
covariance recursion (innovation cholesky LS_t, Kalman gain K_t, filtered
cholesky L_t) is identical for every batch element, so it is computed once on
the host in float64.  The per-batch mean recursion is linear in y:

    mu_f(t)     = mu_pred(t) + K_t (y_t - H mu_pred(t))
    sol(t)      = LS_t^{-1} (y_t - H mu_pred(t))          (whitened innovation)
    mu_pred(t+1)= E mu_f(t),   E = (I + STEP*F)^2

so K=4 consecutive time steps collapse into one matmul over the augmented
input  X_c = [mu_pred ; y_t ; y_t+1 ; y_t+2 ; y_t+3]  (96 x batch).  The
device runs the 32-chunk sequential scan (data-parallel over batch: 64 batch
elements per core x 8 cores) and broadcasts the batch-independent L_t into
the (T, B, 32, 32) output, which dominates the kernel at ~33.5 MB of DMA
writes per core.

log_prob = C - 0.5 * sum_{t,y} sol^2 ; the device returns the per-batch sum
of squares, the constant C is applied on the host.
"""

import numpy as np

B, T, YD, ZD = 512, 128, 16, 32
STEP = 0.05
N_EULER = 2
NCORES = 8
BC = B // NCORES          # 64 batch per core
K = 4                     # time steps per chunk
NCHUNK = T // K           # 32
D = ZD + K * YD           # 96 contraction dim
AUXC = K * ZD + K * YD    # 192 aux matmul cols (mus | sol)

_compiled = {}


# ---------------------------------------------------------------- host math
def _host_precompute(F, H, qdiag, rdiag, sig0diag):
    """Batch-independent covariance recursion in float64."""
    F = F.astype(np.float64)
    H = H.astype(np.float64)
    Q = np.diag(qdiag.astype(np.float64))
    R = np.diag(rdiag.astype(np.float64))
    Iz = np.eye(ZD)
    E1 = Iz + STEP * F
    E = E1 @ E1
    Sig_pred = np.diag(sig0diag.astype(np.float64))
    Ls = np.zeros((T, ZD, ZD))
    Ks = np.zeros((T, ZD, YD))
    LSinvs = np.zeros((T, YD, YD))
    cts = np.zeros(T)
    LOG2PI = np.log(2.0 * np.pi)
    for t in range(T):
        S = H @ Sig_pred @ H.T + R
        LS = np.linalg.cholesky(S)
        PHt = Sig_pred @ H.T
        Kt = np.linalg.solve(S, PHt.T).T
        A = Iz - Kt @ H
        Sig_u = A @ Sig_pred @ A.T + Kt @ R @ Kt.T
        Ls[t] = np.linalg.cholesky(0.5 * (Sig_u + Sig_u.T))
        Ks[t] = Kt
        LSinvs[t] = np.linalg.inv(LS)
        cts[t] = -0.5 * YD * LOG2PI - np.sum(np.log(np.diag(LS)))
        Sig = Sig_u
        for _ in range(N_EULER):
            Sig = Sig + STEP * (F @ Sig + Sig @ F.T + Q)
        Sig_pred = 0.5 * (Sig + Sig.T)

    Wcrit = np.zeros((NCHUNK, D, ZD), np.float32)
    Waux = np.zeros((NCHUNK, D, AUXC), np.float32)
    for c in range(NCHUNK):
        state_map = np.zeros((ZD, D))
        state_map[:, :ZD] = Iz
        mus_maps, sol_maps = [], []
        for j in range(K):
            t = c * K + j
            ymap = np.zeros((YD, D))
            ymap[:, ZD + j * YD: ZD + (j + 1) * YD] = np.eye(YD)
            innov_map = ymap - H @ state_map
            mus_map = state_map + Ks[t] @ innov_map
            sol_maps.append(LSinvs[t] @ innov_map)
            mus_maps.append(mus_map)
            state_map = E @ mus_map
        Wcrit[c] = state_map.T.astype(np.float32)
        Waux[c] = np.concatenate(mus_maps + sol_maps, axis=0).T.astype(np.float32)
    return Ls.astype(np.float32), Wcrit, Waux, float(np.sum(cts))


def _reference_fallback(y, mask, F, H, qdiag, rdiag, mu0, sig0diag):
    """Exact per-batch recursion (float64) for the general-mask case."""
    Bsz = y.shape[0]
    F = F.astype(np.float64); H = H.astype(np.float64)
    Q = np.diag(qdiag.astype(np.float64)); R = np.diag(rdiag.astype(np.float64))
    Iz = np.eye(ZD)
    LOG2PI = np.log(2.0 * np.pi)
    mu_pred = np.broadcast_to(mu0.astype(np.float64), (Bsz, ZD)).copy()
    L_pred = np.broadcast_to(np.diag(np.sqrt(sig0diag.astype(np.float64))),
                             (Bsz, ZD, ZD)).copy()
    mus = np.zeros((T, Bsz, ZD)); Ls = np.zeros((T, Bsz, ZD, ZD))
    logp = np.zeros(Bsz)
    for t in range(T):
        y_i = y[:, t].astype(np.float64); m = mask[:, t].astype(np.float64)
        Sig_pred = L_pred @ np.swapaxes(L_pred, -1, -2)
        S = H @ Sig_pred @ H.T + R
        LS = np.linalg.cholesky(S)
        y_hat = mu_pred @ H.T
        PHt = Sig_pred @ H.T
        Kt = np.swapaxes(np.linalg.solve(S, np.swapaxes(PHt, -1, -2)), -1, -2)
        innov = y_i - y_hat
        mu_u = mu_pred + np.einsum('bzy,by->bz', Kt, innov)
        A = Iz - Kt @ H
        Sig_u = A @ Sig_pred @ np.swapaxes(A, -1, -2) + Kt @ R @ np.swapaxes(Kt, -1, -2)
        L_u = np.linalg.cholesky(0.5 * (Sig_u + np.swapaxes(Sig_u, -1, -2)))
        mu = m[:, None] * mu_u + (1 - m[:, None]) * mu_pred
        L = m[:, None, None] * L_u + (1 - m[:, None, None]) * L_pred
        sol = np.linalg.solve(LS, innov[..., None])[..., 0]
        logp += (-0.5 * YD * LOG2PI
                 - np.sum(np.log(np.diagonal(LS, axis1=-2, axis2=-1)), -1)
                 - 0.5 * np.sum(sol * sol, -1)) * m
        mus[t] = mu; Ls[t] = L
        Sig = L @ np.swapaxes(L, -1, -2)
        mu_n = mu
        for _ in range(N_EULER):
            mu_n = mu_n + STEP * (mu_n @ F.T)
            Sig = Sig + STEP * (F @ Sig + Sig @ F.T + Q)
        mu_pred = mu_n
        L_pred = np.linalg.cholesky(0.5 * (Sig + np.swapaxes(Sig, -1, -2)))
    return (mus.astype(np.float32), Ls.astype(np.float32), logp.astype(np.float32))


# ---------------------------------------------------------------- device
def _build_nc():
    import concourse.bacc as bacc
    import concourse.mybir as mybir
    import concourse.tile as tile

    f32 = mybir.dt.float32
    nc = bacc.Bacc("TRN2", target_bir_lowering=False, debug=False,
                   num_devices=NCORES)

    ybuf = nc.dram_tensor("ybuf", [K * YD, NCHUNK * BC], f32, kind="ExternalInput").ap()
    mu0b = nc.dram_tensor("mu0b", [ZD, BC], f32, kind="ExternalInput").ap()
    wcrit = nc.dram_tensor("wcrit", [D, NCHUNK * ZD], f32, kind="ExternalInput").ap()
    waux = nc.dram_tensor("waux", [D, NCHUNK * AUXC], f32, kind="ExternalInput").ap()
    lsmall = nc.dram_tensor("lsmall", [T, ZD * ZD], f32, kind="ExternalInput").ap()

    ls_out = nc.dram_tensor("ls_out", [T, BC, ZD * ZD], f32, kind="ExternalOutput").ap()
    mus_out = nc.dram_tensor("mus_out", [BC, T * ZD], f32, kind="ExternalOutput").ap()
    ssq_out = nc.dram_tensor("ssq_out", [BC, 1], f32, kind="ExternalOutput").ap()

    with tile.TileContext(nc) as tc:
        with (
            tc.tile_pool(name="const", bufs=1) as constp,
            tc.tile_pool(name="work", bufs=3) as workp,
            tc.tile_pool(name="psumS", bufs=3, space="PSUM") as psumSp,
            tc.tile_pool(name="psumA", bufs=3, space="PSUM") as psumAp,
        ):
            X = constp.tile([D, NCHUNK * BC], f32)
            wc = constp.tile([D, NCHUNK * ZD], f32)
            wa = constp.tile([D, NCHUNK * AUXC], f32)
            ls = constp.tile([T, ZD * ZD], f32)
            musAll = constp.tile([BC, T * ZD], f32)
            acc = constp.tile([BC, K * YD], f32)

            # Broadcast-source loads on the Sync queue right before the
            # broadcasts; scan-critical loads on the Scalar engine's HWDGE
            # queue so they are not FIFO-ordered behind the 33.5 MB
            # broadcast stream.  Inputs are split into smaller DMAs: a
            # single large DMA drains slowly (~100 GB/s) and its completion
            # semaphore stalls later DMAs in the shared rotation.
            # 4 ls loads + 5 scan-input loads = 9 pre-broadcast HWDGE DMAs:
            # fits the sem rotation, so none waits on another and the
            # broadcast stream starts as soon as ls lands (~11us).
            NG = 4
            TG = T // NG
            for g in range(NG):
                nc.sync.dma_start(ls[g * TG:(g + 1) * TG, :],
                                  lsmall[g * TG:(g + 1) * TG, :])
            half = NCHUNK * BC // 2
            nc.scalar.dma_start(X[ZD:D, 0:half], ybuf[:, 0:half])
            nc.scalar.dma_start(X[0:ZD, 0:BC], mu0b)
            nc.scalar.dma_start(X[ZD:D, half:2 * half], ybuf[:, half:2 * half])
            whalf = NCHUNK * ZD // 2
            nc.scalar.dma_start(wc[:, 0:whalf], wcrit[:, 0:whalf])
            nc.scalar.dma_start(wc[:, whalf:2 * whalf], wcrit[:, whalf:2 * whalf])
            # waux on SWDGE: keeps the HWDGE sem rotation clear of slow input
            # completions so the broadcast stream starts without stalls; wa
            # blocks are consumed progressively by the scan, so SWDGE's
            # lower bandwidth is hidden.
            NWA = 12
            wag = NCHUNK * AUXC // NWA
            for g in range(NWA):
                nc.gpsimd.dma_start(wa[:, g * wag:(g + 1) * wag],
                                    waux[:, g * wag:(g + 1) * wag])
            nc.vector.memset(acc[:], 0.0)

            # dominant work: broadcast L_t over the batch dim of ls_out.
            # One dma_start per batch element: separate ring entries fan out
            # across the DMA engines (a single big step-0 DMA runs ~3x
            # slower; per-engine streams cap at ~23 GB/s).
            for b in range(BC):
                nc.sync.dma_start(ls_out[:, b, :], ls[:])

            MCG = 4  # chunks per incremental mus_out DMA
            for c in range(NCHUNK):
                xblk = X[:, c * BC:(c + 1) * BC]
                if c < NCHUNK - 1:
                    ps = psumSp.tile([ZD, BC], f32)
                    nc.tensor.matmul(ps[:], wc[:, c * ZD:(c + 1) * ZD], xblk,
                                     start=True, stop=True)
                    nc.vector.tensor_copy(X[0:ZD, (c + 1) * BC:(c + 2) * BC], ps[:])
                pa = psumAp.tile([BC, AUXC], f32)
                nc.tensor.matmul(pa[:], xblk, wa[:, c * AUXC:(c + 1) * AUXC],
                                 start=True, stop=True)
                nc.scalar.copy(musAll[:, c * K * ZD:(c + 1) * K * ZD], pa[:, 0:K * ZD])
                sq = workp.tile([BC, K * YD], f32)
                nc.scalar.square(sq[:], pa[:, K * ZD:AUXC])
                nc.vector.tensor_add(acc[:], acc[:], sq[:])
                if c % MCG == MCG - 1:
                    # SWDGE: separate semaphore pool, so these mid-stream
                    # writes never stall the HWDGE broadcast rotation.
                    cols = slice((c - MCG + 1) * K * ZD, (c + 1) * K * ZD)
                    nc.gpsimd.dma_start(mus_out[:, cols], musAll[:, cols])
            red = workp.tile([BC, 1], f32)
            nc.vector.reduce_sum(red[:], acc[:], axis=mybir.AxisListType.X)
            nc.gpsimd.dma_start(ssq_out, red[:])

    nc.compile()
    return nc


def _get_nc():
    if "nc" not in _compiled:
        _compiled["nc"] = _build_nc()
    return _compiled["nc"]


LAST_RESULTS = None  # BassKernelResults of the most recent device run


def kernel(y, mask, times, F, H, qdiag, rdiag, mu0, sig0diag):
    global LAST_RESULTS
    y = np.ascontiguousarray(np.asarray(y, np.float32))
    mask = np.asarray(mask, np.float32)
    F = np.asarray(F); H = np.asarray(H)
    qdiag = np.asarray(qdiag); rdiag = np.asarray(rdiag)
    mu0 = np.asarray(mu0); sig0diag = np.asarray(sig0diag)

    if not np.all(mask == 1.0):
        return _reference_fallback(y, mask, F, H, qdiag, rdiag, mu0, sig0diag)

    from concourse.bass_utils import run_bass_kernel_spmd

    Ls, Wcrit, Waux, Ctot = _host_precompute(F, H, qdiag, rdiag, sig0diag)
    wcritA = np.ascontiguousarray(Wcrit.transpose(1, 0, 2).reshape(D, NCHUNK * ZD))
    wauxA = np.ascontiguousarray(Waux.transpose(1, 0, 2).reshape(D, NCHUNK * AUXC))
    lsA = np.ascontiguousarray(Ls.reshape(T, ZD * ZD))
    mu0bA = np.ascontiguousarray(
        np.broadcast_to(mu0.astype(np.float32)[:, None], (ZD, BC)))

    in_maps = []
    for ci in range(NCORES):
        yc = y[ci * BC:(ci + 1) * BC]                       # (BC, T, YD)
        ybuf = np.ascontiguousarray(
            yc.reshape(BC, NCHUNK, K * YD).transpose(2, 1, 0)
            .reshape(K * YD, NCHUNK * BC))                  # X y-rows layout
        in_maps.append(dict(ybuf=ybuf, mu0b=mu0bA, wcrit=wcritA,
                            waux=wauxA, lsmall=lsA))

    nc = _get_nc()
    res = run_bass_kernel_spmd(nc, in_maps, core_ids=list(range(NCORES)))
    LAST_RESULTS = res

    ls_full = np.concatenate(
        [r["ls_out"].reshape(T, BC, ZD, ZD) for r in res.results], axis=1)
    mus_full = np.concatenate(
        [r["mus_out"].reshape(BC, T, ZD) for r in res.results], axis=0
    ).transpose(1, 0, 2)
    ssq = np.concatenate([r["ssq_out"][:, 0] for r in res.results], axis=0)
    logp = (Ctot - 0.5 * ssq.astype(np.float64)).astype(np.float32)
    return mus_full, ls_full, logp
